# revision 23
# baseline (speedup 1.0000x reference)
"""Trainium2 Bass kernel for the CPC loss problem (nn_CPC_85117661872355).

Hybrid strategy (data-parallel over batch B across 8 cores; 1120 rows/core
packed s-major into 9 supergroups of 128 rows):

  Phase 1 (PE): predT[e, r] = sum_d Wk^T[d, e] * ctxT[d, r] as bf16 matmuls
  with the OUTPUT e-dim on partitions; the per-e bias is added in fp32 by the
  scalar engine during the PSUM->SBUF copy.  No repack DMA.

  Phase 2a "G" (supergroups 0-2): SWDGE row-gather of each row's 17 target
  vectors from a bf16 copy of the encoding table; per-row dots via DVE
  tensor_tensor_reduce against a row-major pred tile obtained by PE-transpose
  of predT.  Positives gathered through the same path keep exact ties.

  Phase 2b "F" (supergroups 3-8): full score matrix score = pred_sg @ encT
  on the PE against a per-core ROTATED table (rotation makes the positive
  column index core-independent, so one SPMD program works).  Per-row
  softmax stats come from host-built masks:
    - sum_sel exp = TTR(exp(score) * countmask, add)   (count = multiplicity,
      positive included, so duplicates and pos/neg collisions are exact)
    - max_sel     = TTR(score + addmask, max)          (addmask 0/-300)
    - l0 (positive logit) = diag-mask STT per (s,b) row-block
  All comparisons use the same bf16 score values, so argmax tie semantics
  match the reference exactly.

  Tail: loss = ln(ssum) - negm - pos (negm=0 for F), correct = pos >= maxs,
  both masked by row validity, partition-reduced by a ones-matmul; host sums
  the 8 [1,2] partials.
"""

import functools

import ml_dtypes
import numpy as np

import concourse.bass as bass
import concourse.mybir as mybir
import concourse.tile as tile
from concourse import bacc
from concourse.bass_utils import run_bass_kernel_spmd

F32 = mybir.dt.float32
BF16 = mybir.dt.bfloat16

B, G, D = 64, 7, 1280
S, NEG = 5, 16
NCORES = 8
BSH = B // NCORES  # 8
NT = B * G * G  # 3136 table rows
ROT = BSH * G * G  # 392: per-core table rotation
NS = [BSH * (6 - s) * G for s in range(S)]  # [336, 280, 224, 168, 112]
SOFF = [0]
for n in NS:
    SOFF.append(SOFF[-1] + n)
NR = SOFF[-1]  # 1120 rows per core
NRP = 1152  # padded to 9*128
NSG = 9
G_SGS = [0, 1, 2]
F_SGS = [3, 4, 5, 6, 7, 8]
NDOT = 17
GCHUNKS = [(0, 4), (4, 4), (8, 4), (12, 4), (16, 1)]  # (goff, width)
IDX_PER_SG = NDOT * 128  # 2176
IDX_TOT = len(G_SGS) * IDX_PER_SG
N_PREDS = B * G * 20  # 8960
NTILE = [(i * 512, min(512, NT - i * 512)) for i in range((NT + 511) // 512)]

# row blocks: contiguous (s, b) runs; (row_start, length, pos_col_base)
BLOCKS = []
for s in range(S):
    L = (6 - s) * G
    for b in range(BSH):
        BLOCKS.append((SOFF[s] + b * L, L, b * G * G + (s + 1) * G))


def _sg_blocks(sg):
    """[(pp0, n, cc0)] block slices within supergroup sg."""
    p_lo, p_hi = sg * 128, sg * 128 + 128
    out = []
    for ri0, L, c0 in BLOCKS:
        lo, hi = max(ri0, p_lo), min(ri0 + L, p_hi)
        if lo < hi:
            out.append((lo - p_lo, hi - lo, c0 + (lo - ri0)))
    return out


LAST_RUN = {}


@functools.lru_cache(maxsize=1)
def build_nc() -> bass.Bass:
    nc = bacc.Bacc(
        "TRN2",
        target_bir_lowering=False,
        debug=False,
        num_devices=NCORES,
    )
    ctxTh = nc.declare_dram_parameter("ctxTh", [D, NR], BF16, isOutput=False)
    wkTh = nc.declare_dram_parameter("wkTh", [S, D, D], BF16, isOutput=False)
    wkbH = nc.declare_dram_parameter("wkbH", [1, S, D], BF16, isOutput=False)
    ench = nc.declare_dram_parameter("ench", [NT, D], BF16, isOutput=False)
    encT = nc.declare_dram_parameter("encT", [D, NT], BF16, isOutput=False)
    cmask = nc.declare_dram_parameter("cmask", [len(F_SGS), 128, NT], BF16, isOutput=False)
    amask = nc.declare_dram_parameter("amask", [len(F_SGS), 128, NT], BF16, isOutput=False)
    dmask = nc.declare_dram_parameter("dmask", [len(F_SGS), 128, ROT], BF16, isOutput=False)
    ident = nc.declare_dram_parameter("ident", [128, 128], BF16, isOutput=False)
    idx = nc.declare_dram_parameter(
        "idx", [128, IDX_TOT // 16], mybir.dt.int16, isOutput=False
    )
    out = nc.declare_dram_parameter("out", [1, 2], F32, isOutput=True)

    Alu = mybir.AluOpType
    Act = mybir.ActivationFunctionType
    Ax = mybir.AxisListType

    with tile.TileContext(nc) as tc:
        with (
            tc.tile_pool(name="const", bufs=1) as constp,
            tc.tile_pool(name="wk", bufs=2) as wkp,
            tc.tile_pool(name="gath", bufs=2) as gathp,
            tc.tile_pool(name="score", bufs=2) as scorep,
            tc.tile_pool(name="mask", bufs=1) as maskp,
            tc.tile_pool(name="scr", bufs=1) as scrp,
            tc.tile_pool(name="dots", bufs=2) as dotsp,
            tc.tile_pool(name="small", bufs=4) as smallp,
            tc.tile_pool(name="psum", bufs=3, space="PSUM") as psump,
            tc.tile_pool(name="psumS", bufs=2, space="PSUM") as psumSp,
            tc.tile_pool(name="psumT", bufs=2, space="PSUM") as psumTp,
            tc.tile_pool(name="psumf", bufs=1, space="PSUM") as psumfp,
        ):
            # ---- constants ----
            idx_sb = constp.tile([128, IDX_TOT // 16], mybir.dt.int16, tag="idx")
            nc.sync.dma_start(idx_sb[:, :], idx[:, :])
            ident_sb = constp.tile([128, 128], BF16, tag="ident")
            nc.sync.dma_start(ident_sb[:, :], ident[:, :])
            dmask_sb = constp.tile([128, len(F_SGS), ROT], BF16, tag="dmask")
            nc.sync.dma_start(
                dmask_sb[:, :, :], dmask[:, :, :].rearrange("f p c -> p f c")
            )
            wkbh_sb = constp.tile([1, S, D], BF16, tag="wkbh")
            nc.sync.dma_start(wkbh_sb[:, :, :], wkbH[:, :, :])
            ones_sb = constp.tile([128, 1], F32, tag="ones")
            nc.vector.memset(ones_sb[:, :], 1.0)
            onesb16 = constp.tile([1, 512], BF16, tag="onesb16")
            nc.vector.memset(onesb16[:, :], 1.0)
            ctxh_sb = constp.tile([128, 10, NR], BF16, tag="ctxh")
            nc.sync.dma_start(
                ctxh_sb[:, :, :], ctxTh[:, :].rearrange("(dc di) r -> di dc r", di=128)
            )
            encT_sb = constp.tile([128, 10, NT], BF16, tag="encT")
            nc.sync.dma_start(
                encT_sb[:, :, :], encT[:, :].rearrange("(dc di) n -> di dc n", di=128)
            )

            # CE partial columns
            negm_all = constp.tile([128, NSG], F32, tag="negm_all")
            ssum_all = constp.tile([128, NSG], F32, tag="ssum_all")
            pos_all = constp.tile([128, NSG], F32, tag="pos_all")
            maxs_all = constp.tile([128, NSG], F32, tag="maxs_all")
            nc.vector.memset(negm_all[:, :], 0.0)
            nc.vector.memset(pos_all[:, :], 0.0)
            vmask = constp.tile([128, NSG], F32, tag="vmask")
            nc.vector.memset(vmask[:, :], 1.0)
            nc.vector.memset(vmask[96:128, 8:9], 0.0)

            # transposed predictions (lhsT for F-path); pad cols zeroed
            predT = constp.tile([128, 10, NRP], BF16, tag="predT")
            nc.vector.memset(predT[:, :, NR:NRP], 0.0)
            pred_g = [
                constp.tile([128, D], BF16, tag="predg", name=f"predg{i}")
                for i in G_SGS
            ]

            # ---- phase 1: predT = Wk^T-slices @ ctxT (+bias on copy) ----
            for s in range(S):
                wk_r = wkTh[s, :, :].rearrange("(dc di) e -> di dc e", di=128)
                for ch2 in range(5):  # 2 e-chunks of 128 per wk tile
                    wk_t = wkp.tile([128, 10, 256], BF16, tag="wk")
                    nc.sync.dma_start(
                        wk_t[:, :, :], wk_r[:, :, ch2 * 256 : ch2 * 256 + 256]
                    )
                    for e2 in range(2):
                        ec = ch2 * 2 + e2
                        ps = psump.tile([128, NS[s]], F32, tag="ps")
                        for dc in range(10):
                            nc.tensor.matmul(
                                ps[:, :],
                                lhsT=wk_t[:, dc, e2 * 128 : e2 * 128 + 128],
                                rhs=ctxh_sb[:, dc, SOFF[s] : SOFF[s] + NS[s]],
                                start=(dc == 0),
                                stop=False,
                            )
                        # bias: out[e, r] += wkb[e] * 1
                        nc.tensor.matmul(
                            ps[:, :],
                            lhsT=wkbh_sb[0:1, s, ec * 128 : ec * 128 + 128],
                            rhs=onesb16[0:1, : NS[s]],
                            start=False,
                            stop=True,
                        )
                        nc.scalar.copy(
                            predT[:, ec, SOFF[s] : SOFF[s] + NS[s]], ps[:, :]
                        )
                # after s=0: rows 0..335 ready -> G sgs 0,1; after s=1: sg 2
                ready = {0: [0, 1], 1: [2]}.get(s, [])
                for gi in ready:
                    for dc in range(10):
                        pt = psumTp.tile([128, 128], BF16, tag="pt")
                        nc.tensor.transpose(
                            pt[:, :],
                            predT[:, dc, gi * 128 : gi * 128 + 128],
                            ident_sb[:, :],
                        )
                        nc.vector.tensor_copy(
                            pred_g[gi][:, dc * 128 : dc * 128 + 128], pt[:, :]
                        )

            # ---- phase 2a (G): gather + TTR dots + CE partials ----
            ench_ap = ench[:, :]
            for gi, sg in enumerate(G_SGS):
                dots_t = dotsp.tile([128, NDOT], F32, tag="dots")
                for goff, w in GCHUNKS:
                    gt = gathp.tile([128, 4, D], BF16, tag="gt")
                    pos0 = gi * IDX_PER_SG + goff * 128
                    nidx = w * 128
                    nc.gpsimd.dma_gather(
                        gt[:, :w, :],
                        ench_ap,
                        idx_sb[:, pos0 // 16 : (pos0 + nidx) // 16],
                        nidx,
                        nidx,
                        D,
                    )
                    for j in range(w):
                        scr = scrp.tile([128, NT], BF16, tag="fscr")
                        g = goff + j
                        nc.vector.scalar_tensor_tensor(
                            scr[:, 0:D],
                            gt[:, j, :],
                            1.0,
                            pred_g[gi][:, :],
                            op0=Alu.mult,
                            op1=Alu.mult,
                            accum_out=dots_t[:, g : g + 1],
                        )
                nc.vector.tensor_reduce(
                    negm_all[:, sg : sg + 1], dots_t[:, :], Ax.X, Alu.max, negate=True
                )
                e17 = smallp.tile([128, NDOT], F32, tag="e17")
                nc.scalar.activation(
                    e17[:, :],
                    dots_t[:, :],
                    Act.Exp,
                    bias=negm_all[:, sg : sg + 1],
                    scale=1.0,
                    accum_out=ssum_all[:, sg : sg + 1],
                )
                nc.vector.tensor_reduce(
                    maxs_all[:, sg : sg + 1], dots_t[:, 1:NDOT], Ax.X, Alu.max
                )
                nc.vector.tensor_copy(pos_all[:, sg : sg + 1], dots_t[:, 0:1])

            # ---- phase 2b (F): PE score matrix + mask reductions ----
            for fi, sg in enumerate(F_SGS):
                am = maskp.tile([128, NT], BF16, tag="am")
                nc.sync.dma_start(am[:, :], amask[fi, :, :])
                cm = maskp.tile([128, NT], BF16, tag="cm")
                nc.sync.dma_start(cm[:, :], cmask[fi, :, :])
                score = scorep.tile([128, NT], BF16, tag="score")
                for n0, nw in NTILE:
                    psS = psumSp.tile([128, 512], F32, tag="psS")
                    for dc in range(10):
                        nc.tensor.matmul(
                            psS[:, :nw],
                            lhsT=predT[:, dc, sg * 128 : sg * 128 + 128],
                            rhs=encT_sb[:, dc, n0 : n0 + nw],
                            start=(dc == 0),
                            stop=(dc == 9),
                        )
                    nc.scalar.copy(score[:, n0 : n0 + nw], psS[:, :nw])
                scrF = scrp.tile([128, NT], BF16, tag="fscr")
                nc.vector.tensor_tensor(scrF[:, :], score[:, :], am[:, :], Alu.add)
                nc.vector.tensor_reduce(
                    maxs_all[:, sg : sg + 1], scrF[:, :], Ax.X, Alu.max
                )
                e_t = scrp.tile([128, NT], BF16, tag="e_t")
                nc.scalar.activation(e_t[:, :], score[:, :], Act.Exp)
                nc.vector.scalar_tensor_tensor(
                    scrF[:, :],
                    e_t[:, :],
                    1.0,
                    cm[:, :],
                    op0=Alu.mult,
                    op1=Alu.mult,
                    accum_out=ssum_all[:, sg : sg + 1],
                )
                # positive logit: one-hot select over the [0, ROT) col window
                scrD = scrp.tile([128, NT], BF16, tag="fscr")
                nc.vector.scalar_tensor_tensor(
                    scrD[:, 0:ROT],
                    score[:, 0:ROT],
                    1.0,
                    dmask_sb[:, fi, :],
                    op0=Alu.mult,
                    op1=Alu.mult,
                    accum_out=pos_all[:, sg : sg + 1],
                )

            # ---- batched CE tail ----
            lns = smallp.tile([128, NSG], F32, tag="lns")
            nc.scalar.activation(lns[:, :], ssum_all[:, :], Act.Ln)
            loss_t = smallp.tile([128, NSG], F32, tag="loss_t")
            nc.vector.tensor_tensor(loss_t[:, :], lns[:, :], negm_all[:, :], Alu.subtract)
            nc.vector.tensor_tensor(loss_t[:, :], loss_t[:, :], pos_all[:, :], Alu.subtract)
            nc.vector.tensor_tensor(loss_t[:, :], loss_t[:, :], vmask[:, :], Alu.mult)
            acc2 = smallp.tile([128, 2], F32, tag="acc2")
            nc.vector.tensor_reduce(acc2[:, 0:1], loss_t[:, :], Ax.X, Alu.add)
            corr_t = smallp.tile([128, NSG], F32, tag="corr_t")
            nc.vector.tensor_tensor(corr_t[:, :], pos_all[:, :], maxs_all[:, :], Alu.is_ge)
            nc.vector.tensor_tensor(corr_t[:, :], corr_t[:, :], vmask[:, :], Alu.mult)
            nc.vector.tensor_reduce(acc2[:, 1:2], corr_t[:, :], Ax.X, Alu.add)

            psf = psumfp.tile([1, 2], F32, tag="psf")
            nc.tensor.matmul(
                psf[:, :], lhsT=ones_sb[:, 0:1], rhs=acc2[:, :], start=True, stop=True
            )
            outsb = smallp.tile([1, 2], F32, tag="outsb")
            nc.vector.tensor_copy(outsb[:, :], psf[:, :])
            nc.sync.dma_start(out[:, :], outsb[:, :])

    nc.compile()
    return nc


def _row_targets(core: int, neg_idx: np.ndarray, nrows: int) -> np.ndarray:
    """[nrows, 17] flat enc index (unrotated) of positive + negatives."""
    tg = np.zeros((nrows, NDOT), np.int64)
    ri = 0
    for s in range(S):
        rows = 6 - s
        for b in range(BSH):
            bg = core * BSH + b
            for r in range(rows):
                for c7 in range(G):
                    if ri >= nrows:
                        return tg
                    tg[ri, 0] = bg * G * G + (s + 1 + r) * G + c7
                    tg[ri, 1:] = neg_idx[bg, s, r, c7]
                    ri += 1
    return tg


def _build_idx(core: int, neg_idx: np.ndarray) -> np.ndarray:
    """int16 gather-index tensor (G supergroups only) in SWDGE wrap layout."""
    tg = _row_targets(core, neg_idx, len(G_SGS) * 128)
    lst = tg.reshape(len(G_SGS), 128, NDOT).transpose(0, 2, 1).reshape(-1)
    arr = lst.astype(np.int16).reshape(-1, 16).T  # [16, IDX_TOT//16]
    return np.ascontiguousarray(np.tile(arr, (8, 1)))


def _build_masks(core: int, neg_idx: np.ndarray):
    """count / additive masks [6, 128, NT] (rotated cols) + diag mask."""
    tg_all = _row_targets(core, neg_idx, NR)  # unrotated
    tg_rot = (tg_all - core * ROT) % NT
    cnt = np.zeros((len(F_SGS), 128, NT), np.float32)
    add = np.full((len(F_SGS), 128, NT), -300.0, np.float32)
    dmk = np.zeros((len(F_SGS), 128, ROT), np.float32)
    for fi, sg in enumerate(F_SGS):
        for p in range(128):
            ri = sg * 128 + p
            if ri < NR:
                np.add.at(cnt[fi, p], tg_rot[ri], 1.0)
                add[fi, p, tg_rot[ri]] = 0.0
                assert tg_rot[ri, 0] < ROT
                dmk[fi, p, tg_rot[ri, 0]] = 1.0
            else:  # pad row: benign singleton at col 0
                cnt[fi, p, 0] = 1.0
                add[fi, p, 0] = 0.0
    return (
        cnt.astype(ml_dtypes.bfloat16),
        add.astype(ml_dtypes.bfloat16),
        dmk.astype(ml_dtypes.bfloat16),
    )


def _prep_in_maps(contexts, encodings, Wk_w, Wk_b, neg_idx):
    contexts = np.ascontiguousarray(np.asarray(contexts, np.float32))
    encodings = np.ascontiguousarray(np.asarray(encodings, np.float32))
    Wk_w = np.ascontiguousarray(np.asarray(Wk_w, np.float32))
    Wk_b = np.ascontiguousarray(np.asarray(Wk_b, np.float32))
    neg_idx = np.asarray(neg_idx)

    enc_flat = encodings.reshape(NT, D)
    ench = np.ascontiguousarray(enc_flat.astype(ml_dtypes.bfloat16))
    wkT = Wk_w.transpose(0, 2, 1)  # [S, d, e]
    wkTh = np.ascontiguousarray(wkT.astype(ml_dtypes.bfloat16))
    wkbH = np.ascontiguousarray(Wk_b[None, :, :].astype(ml_dtypes.bfloat16))
    identm = np.ascontiguousarray(np.eye(128, dtype=ml_dtypes.bfloat16))

    in_maps = []
    for c in range(NCORES):
        bs = slice(c * BSH, (c + 1) * BSH)
        ctx_rows = np.concatenate(
            [contexts[bs, : 6 - s].reshape(-1, D) for s in range(S)], axis=0
        )
        ctxTh = ctx_rows.T.astype(ml_dtypes.bfloat16)
        enc_rot = np.roll(enc_flat, -c * ROT, axis=0)
        encTc = np.ascontiguousarray(enc_rot.T.astype(ml_dtypes.bfloat16))
        cnt, add, dmk = _build_masks(c, neg_idx)
        in_maps.append(
            {
                "ctxTh": np.ascontiguousarray(ctxTh),
                "wkTh": wkTh,
                "wkbH": wkbH,
                "ench": ench,
                "encT": encTc,
                "cmask": np.ascontiguousarray(cnt),
                "amask": np.ascontiguousarray(add),
                "dmask": np.ascontiguousarray(dmk),
                "ident": identm,
                "idx": _build_idx(c, neg_idx),
            }
        )
    return in_maps


def kernel(contexts, encodings, Wk_w, Wk_b, neg_idx, _trace=False):
    in_maps = _prep_in_maps(contexts, encodings, Wk_w, Wk_b, neg_idx)
    nc = build_nc()
    res = run_bass_kernel_spmd(nc, in_maps, list(range(NCORES)), trace=_trace)
    LAST_RUN["exec_time_ns"] = res.exec_time_ns
    LAST_RUN["results"] = res.results
    loss = np.float32(0.0)
    corr = np.float32(0.0)
    for o in res.results:
        loss += np.float32(o["out"][0, 0])
        corr += np.float32(o["out"][0, 1])
    return (
        np.float32(loss / np.float32(N_PREDS)),
        np.float32(corr / np.float32(N_PREDS)),
    )


# revision 30
# speedup vs baseline: 1.3332x; 1.3332x over previous
"""Trainium2 Bass kernel for the CPC loss problem (nn_CPC_85117661872355).

Hybrid strategy (data-parallel over batch B across 8 cores; 1120 rows/core
packed s-major into 9 supergroups of 128 rows):

  Phase 1 (PE): predT[e, r] = sum_d Wk^T[d, e] * ctxT[d, r] as bf16 matmuls
  with the OUTPUT e-dim on partitions; the per-e bias is added in fp32 by the
  scalar engine during the PSUM->SBUF copy.  No repack DMA.

  Phase 2a "G" (supergroups 0-2): SWDGE row-gather of each row's 17 target
  vectors from a bf16 copy of the encoding table; per-row dots via DVE
  tensor_tensor_reduce against a row-major pred tile obtained by PE-transpose
  of predT.  Positives gathered through the same path keep exact ties.

  Phase 2b "F" (supergroups 3-8): full score matrix score = pred_sg @ encT
  on the PE against a per-core ROTATED table (rotation makes the positive
  column index core-independent, so one SPMD program works).  Per-row
  softmax stats come from host-built masks:
    - sum_sel exp = TTR(exp(score) * countmask, add)   (count = multiplicity,
      positive included, so duplicates and pos/neg collisions are exact)
    - max_sel     = TTR(score + addmask, max)          (addmask 0/-300)
    - l0 (positive logit) = diag-mask STT per (s,b) row-block
  All comparisons use the same bf16 score values, so argmax tie semantics
  match the reference exactly.

  Tail: loss = ln(ssum) - negm - pos (negm=0 for F), correct = pos >= maxs,
  both masked by row validity, partition-reduced by a ones-matmul; host sums
  the 8 [1,2] partials.
"""

import functools

import ml_dtypes
import numpy as np

import concourse.bass as bass
import concourse.mybir as mybir
import concourse.tile as tile
from concourse import bacc
from concourse.bass_utils import run_bass_kernel_spmd

F32 = mybir.dt.float32
BF16 = mybir.dt.bfloat16

B, G, D = 64, 7, 1280
S, NEG = 5, 16
NCORES = 8
BSH = B // NCORES  # 8
NT = B * G * G  # 3136 table rows
ROT = BSH * G * G  # 392: per-core table rotation
NS = [BSH * (6 - s) * G for s in range(S)]  # [336, 280, 224, 168, 112]
SOFF = [0]
for n in NS:
    SOFF.append(SOFF[-1] + n)
NR = SOFF[-1]  # 1120 rows per core
NRP = 1152  # padded to 9*128
NSG = 9
G_SGS = [0, 1, 2]
F_SGS = [3, 4, 5, 6, 7, 8]
NDOT = 17
GCHUNKS = [(0, 4), (4, 4), (8, 4), (12, 4), (16, 1)]  # (goff, width)
IDX_PER_SG = NDOT * 128  # 2176
IDX_TOT = len(G_SGS) * IDX_PER_SG
N_PREDS = B * G * 20  # 8960
NTILE = [(i * 512, min(512, NT - i * 512)) for i in range((NT + 511) // 512)]

# row blocks: contiguous (s, b) runs; (row_start, length, pos_col_base)
BLOCKS = []
for s in range(S):
    L = (6 - s) * G
    for b in range(BSH):
        BLOCKS.append((SOFF[s] + b * L, L, b * G * G + (s + 1) * G))


def _sg_blocks(sg):
    """[(pp0, n, cc0)] block slices within supergroup sg."""
    p_lo, p_hi = sg * 128, sg * 128 + 128
    out = []
    for ri0, L, c0 in BLOCKS:
        lo, hi = max(ri0, p_lo), min(ri0 + L, p_hi)
        if lo < hi:
            out.append((lo - p_lo, hi - lo, c0 + (lo - ri0)))
    return out


LAST_RUN = {}


@functools.lru_cache(maxsize=1)
def build_nc() -> bass.Bass:
    nc = bacc.Bacc(
        "TRN2",
        target_bir_lowering=False,
        debug=False,
        num_devices=NCORES,
    )
    ctxTh = nc.declare_dram_parameter("ctxTh", [D, NR], BF16, isOutput=False)
    wkTh = nc.declare_dram_parameter("wkTh", [S, D, D], BF16, isOutput=False)
    wkbH = nc.declare_dram_parameter("wkbH", [1, S, D], BF16, isOutput=False)
    ench = nc.declare_dram_parameter("ench", [NT, D], BF16, isOutput=False)
    encT = nc.declare_dram_parameter("encT", [D, NT], BF16, isOutput=False)
    cmask = nc.declare_dram_parameter("cmask", [len(F_SGS), 128, NT], BF16, isOutput=False)
    amask = nc.declare_dram_parameter("amask", [len(F_SGS), 128, NT], BF16, isOutput=False)
    dmask = nc.declare_dram_parameter("dmask", [len(F_SGS), 128, ROT], BF16, isOutput=False)
    idx = nc.declare_dram_parameter(
        "idx", [128, IDX_TOT // 16], mybir.dt.int16, isOutput=False
    )
    out = nc.declare_dram_parameter("out", [1, 2], F32, isOutput=True)

    Alu = mybir.AluOpType
    Act = mybir.ActivationFunctionType
    Ax = mybir.AxisListType

    with tile.TileContext(nc) as tc:
        with (
            tc.tile_pool(name="const", bufs=1) as constp,
            tc.tile_pool(name="wk", bufs=2) as wkp,
            tc.tile_pool(name="gath", bufs=2) as gathp,
            tc.tile_pool(name="predg", bufs=3) as predgp,
            tc.tile_pool(name="score", bufs=2) as scorep,
            tc.tile_pool(name="mask", bufs=1) as maskp,
            tc.tile_pool(name="scr", bufs=1) as scrp,
            tc.tile_pool(name="dots", bufs=2) as dotsp,
            tc.tile_pool(name="small", bufs=4) as smallp,
            tc.tile_pool(name="stage", bufs=2) as stagep,
            tc.tile_pool(name="psum", bufs=2, space="PSUM") as psump,
            tc.tile_pool(name="psumG", bufs=2, space="PSUM") as psumGp,
            tc.tile_pool(name="psumS", bufs=3, space="PSUM") as psumSp,
            tc.tile_pool(name="psumf", bufs=1, space="PSUM") as psumfp,
        ):
            # ---- constants ----
            idx_sb = constp.tile([128, IDX_TOT // 16], mybir.dt.int16, tag="idx")
            nc.sync.dma_start(idx_sb[:, :], idx[:, :])
            dmask_sb = constp.tile([128, len(F_SGS), ROT], BF16, tag="dmask")
            nc.sync.dma_start(
                dmask_sb[:, :, :], dmask[:, :, :].rearrange("f p c -> p f c")
            )
            wkbh_sb = constp.tile([1, S, D], BF16, tag="wkbh")
            nc.sync.dma_start(wkbh_sb[:, :, :], wkbH[:, :, :])
            ones_sb = constp.tile([128, 1], F32, tag="ones")
            nc.vector.memset(ones_sb[:, :], 1.0)
            onesb16 = constp.tile([1, 512], BF16, tag="onesb16")
            nc.vector.memset(onesb16[:, :], 1.0)
            ctxh_sb = constp.tile([128, 10, NR], BF16, tag="ctxh")
            nc.sync.dma_start(
                ctxh_sb[:, :, :], ctxTh[:, :].rearrange("(dc di) r -> di dc r", di=128)
            )
            encT_sb = constp.tile([128, 10, NT], BF16, tag="encT")
            nc.sync.dma_start(
                encT_sb[:, :, :], encT[:, :].rearrange("(dc di) n -> di dc n", di=128)
            )

            # CE partial columns
            negm_all = constp.tile([128, NSG], F32, tag="negm_all")
            ssum_all = constp.tile([128, NSG], F32, tag="ssum_all")
            pos_all = constp.tile([128, NSG], F32, tag="pos_all")
            maxs_all = constp.tile([128, NSG], F32, tag="maxs_all")
            nc.vector.memset(negm_all[:, :], 0.0)
            nc.vector.memset(pos_all[:, :], 0.0)
            vmask = constp.tile([128, NSG], F32, tag="vmask")
            nc.vector.memset(vmask[:, :], 1.0)
            nc.vector.memset(vmask[96:128, 8:9], 0.0)

            # transposed predictions (lhsT for F-path); pad cols zeroed
            predT = constp.tile([128, 10, NRP], BF16, tag="predT")
            nc.vector.memset(predT[:, :, NR:NRP], 0.0)
            pred_g = [
                predgp.tile([128, D], BF16, tag="predg", name=f"predg{i}")
                for i in G_SGS
            ]

            # ---- phase 1: predT = Wk^T-slices @ ctxT (+bias on copy) ----
            for s in range(S):
                wk_r = wkTh[s, :, :].rearrange("(dc di) e -> di dc e", di=128)
                for ch2 in range(5):  # 2 e-chunks of 128 per wk tile
                    wk_t = wkp.tile([128, 10, 256], BF16, tag="wk")
                    nc.sync.dma_start(
                        wk_t[:, :, :], wk_r[:, :, ch2 * 256 : ch2 * 256 + 256]
                    )
                    for e2 in range(2):
                        ec = ch2 * 2 + e2
                        ps = psump.tile([128, NS[s]], F32, tag="ps")
                        for dc in range(10):
                            nc.tensor.matmul(
                                ps[:, :],
                                lhsT=wk_t[:, dc, e2 * 128 : e2 * 128 + 128],
                                rhs=ctxh_sb[:, dc, SOFF[s] : SOFF[s] + NS[s]],
                                start=(dc == 0),
                                stop=False,
                            )
                        # bias: out[e, r] += wkb[e] * 1
                        nc.tensor.matmul(
                            ps[:, :],
                            lhsT=wkbh_sb[0:1, s, ec * 128 : ec * 128 + 128],
                            rhs=onesb16[0:1, : NS[s]],
                            start=False,
                            stop=True,
                        )
                        nc.scalar.copy(
                            predT[:, ec, SOFF[s] : SOFF[s] + NS[s]], ps[:, :]
                        )
                    # row-major pred for the G supergroups (rows 0..383):
                    # baseline-proven matmul + stage + repack-DMA pattern
                    for roff, M in {0: [(0, 128), (128, 128), (256, 80)],
                                    1: [(336, 48)]}.get(s, []):
                        psG = psumGp.tile([128, 256], F32, tag="psG")
                        for dc in range(10):
                            nc.tensor.matmul(
                                psG[:M, :],
                                lhsT=ctxh_sb[:, dc, roff : roff + M],
                                rhs=wk_t[:, dc, :],
                                start=(dc == 0),
                                stop=False,
                            )
                        nc.tensor.matmul(
                            psG[:M, :],
                            lhsT=onesb16[0:1, :M],
                            rhs=wkbh_sb[0:1, s, ch2 * 256 : ch2 * 256 + 256],
                            start=False,
                            stop=True,
                        )
                        stg = stagep.tile([128, 256], BF16, tag="stg")
                        nc.scalar.copy(stg[:M, :], psG[:M, :])
                        k, p0 = divmod(roff, 128)
                        n1 = min(M, 128 - p0)
                        nc.sync.dma_start(
                            pred_g[k][p0 : p0 + n1, ch2 * 256 : ch2 * 256 + 256],
                            stg[0:n1, :],
                        )
                        if M > n1:
                            nc.sync.dma_start(
                                pred_g[k + 1][0 : M - n1, ch2 * 256 : ch2 * 256 + 256],
                                stg[n1:M, :],
                            )

            # ---- phase 2a (G): gather + TTR dots + CE partials ----
            ench_ap = ench[:, :]
            for gi, sg in enumerate(G_SGS):
                dots_t = dotsp.tile([128, NDOT], F32, tag="dots")
                for goff, w in GCHUNKS:
                    gt = gathp.tile([128, 4, D], BF16, tag="gt")
                    pos0 = gi * IDX_PER_SG + goff * 128
                    nidx = w * 128
                    nc.gpsimd.dma_gather(
                        gt[:, :w, :],
                        ench_ap,
                        idx_sb[:, pos0 // 16 : (pos0 + nidx) // 16],
                        nidx,
                        nidx,
                        D,
                    )
                    for j in range(w):
                        scr = scrp.tile([128, NT], BF16, tag="fscr")
                        g = goff + j
                        nc.vector.scalar_tensor_tensor(
                            scr[:, 0:D],
                            gt[:, j, :],
                            1.0,
                            pred_g[gi][:, :],
                            op0=Alu.mult,
                            op1=Alu.mult,
                            accum_out=dots_t[:, g : g + 1],
                        )
                nc.vector.tensor_reduce(
                    negm_all[:, sg : sg + 1], dots_t[:, :], Ax.X, Alu.max, negate=True
                )
                e17 = smallp.tile([128, NDOT], F32, tag="e17")
                nc.scalar.activation(
                    e17[:, :],
                    dots_t[:, :],
                    Act.Exp,
                    bias=negm_all[:, sg : sg + 1],
                    scale=1.0,
                    accum_out=ssum_all[:, sg : sg + 1],
                )
                nc.vector.tensor_reduce(
                    maxs_all[:, sg : sg + 1], dots_t[:, 1:NDOT], Ax.X, Alu.max
                )
                nc.vector.tensor_copy(pos_all[:, sg : sg + 1], dots_t[:, 0:1])

            # ---- phase 2b (F): PE score matrix + mask reductions ----
            for fi, sg in enumerate(F_SGS):
                am = maskp.tile([128, NT], BF16, tag="am")
                nc.sync.dma_start(am[:, :], amask[fi, :, :])
                cm = maskp.tile([128, NT], BF16, tag="cm")
                nc.sync.dma_start(cm[:, :], cmask[fi, :, :])
                score = scorep.tile([128, NT], BF16, tag="score")
                for n0, nw in NTILE:
                    psS = psumSp.tile([128, 512], F32, tag="psS")
                    for dc in range(10):
                        nc.tensor.matmul(
                            psS[:, :nw],
                            lhsT=predT[:, dc, sg * 128 : sg * 128 + 128],
                            rhs=encT_sb[:, dc, n0 : n0 + nw],
                            start=(dc == 0),
                            stop=(dc == 9),
                        )
                    nc.scalar.copy(score[:, n0 : n0 + nw], psS[:, :nw])
                scrF = scrp.tile([128, NT], BF16, tag="fscr")
                nc.vector.tensor_tensor(scrF[:, :], score[:, :], am[:, :], Alu.add)
                nc.vector.tensor_reduce(
                    maxs_all[:, sg : sg + 1], scrF[:, :], Ax.X, Alu.max
                )
                e_t = scrp.tile([128, NT], BF16, tag="e_t")
                nc.scalar.activation(e_t[:, :], score[:, :], Act.Exp)
                nc.vector.scalar_tensor_tensor(
                    scrF[:, :],
                    e_t[:, :],
                    1.0,
                    cm[:, :],
                    op0=Alu.mult,
                    op1=Alu.mult,
                    accum_out=ssum_all[:, sg : sg + 1],
                )
                # positive logit: one-hot select over the [0, ROT) col window
                scrD = scrp.tile([128, NT], BF16, tag="fscr")
                nc.vector.scalar_tensor_tensor(
                    scrD[:, 0:ROT],
                    score[:, 0:ROT],
                    1.0,
                    dmask_sb[:, fi, :],
                    op0=Alu.mult,
                    op1=Alu.mult,
                    accum_out=pos_all[:, sg : sg + 1],
                )

            # ---- batched CE tail ----
            lns = smallp.tile([128, NSG], F32, tag="lns")
            nc.scalar.activation(lns[:, :], ssum_all[:, :], Act.Ln)
            loss_t = smallp.tile([128, NSG], F32, tag="loss_t")
            nc.vector.tensor_tensor(loss_t[:, :], lns[:, :], negm_all[:, :], Alu.subtract)
            nc.vector.tensor_tensor(loss_t[:, :], loss_t[:, :], pos_all[:, :], Alu.subtract)
            nc.vector.tensor_tensor(loss_t[:, :], loss_t[:, :], vmask[:, :], Alu.mult)
            acc2 = smallp.tile([128, 2], F32, tag="acc2")
            nc.vector.tensor_reduce(acc2[:, 0:1], loss_t[:, :], Ax.X, Alu.add)
            corr_t = smallp.tile([128, NSG], F32, tag="corr_t")
            nc.vector.tensor_tensor(corr_t[:, :], pos_all[:, :], maxs_all[:, :], Alu.is_ge)
            nc.vector.tensor_tensor(corr_t[:, :], corr_t[:, :], vmask[:, :], Alu.mult)
            nc.vector.tensor_reduce(acc2[:, 1:2], corr_t[:, :], Ax.X, Alu.add)

            psf = psumfp.tile([1, 2], F32, tag="psf")
            nc.tensor.matmul(
                psf[:, :], lhsT=ones_sb[:, 0:1], rhs=acc2[:, :], start=True, stop=True
            )
            outsb = smallp.tile([1, 2], F32, tag="outsb")
            nc.vector.tensor_copy(outsb[:, :], psf[:, :])
            nc.sync.dma_start(out[:, :], outsb[:, :])

    nc.compile()
    return nc


def _row_targets(core: int, neg_idx: np.ndarray, nrows: int) -> np.ndarray:
    """[nrows, 17] flat enc index (unrotated) of positive + negatives."""
    tg = np.zeros((nrows, NDOT), np.int64)
    ri = 0
    for s in range(S):
        rows = 6 - s
        for b in range(BSH):
            bg = core * BSH + b
            for r in range(rows):
                for c7 in range(G):
                    if ri >= nrows:
                        return tg
                    tg[ri, 0] = bg * G * G + (s + 1 + r) * G + c7
                    tg[ri, 1:] = neg_idx[bg, s, r, c7]
                    ri += 1
    return tg


def _build_idx(core: int, neg_idx: np.ndarray) -> np.ndarray:
    """int16 gather-index tensor (G supergroups only) in SWDGE wrap layout."""
    tg = _row_targets(core, neg_idx, len(G_SGS) * 128)
    lst = tg.reshape(len(G_SGS), 128, NDOT).transpose(0, 2, 1).reshape(-1)
    arr = lst.astype(np.int16).reshape(-1, 16).T  # [16, IDX_TOT//16]
    return np.ascontiguousarray(np.tile(arr, (8, 1)))


def _build_masks(core: int, neg_idx: np.ndarray):
    """count / additive masks [6, 128, NT] (rotated cols) + diag mask."""
    tg_all = _row_targets(core, neg_idx, NR)  # unrotated
    tg_rot = (tg_all - core * ROT) % NT
    cnt = np.zeros((len(F_SGS), 128, NT), np.float32)
    add = np.full((len(F_SGS), 128, NT), -300.0, np.float32)
    dmk = np.zeros((len(F_SGS), 128, ROT), np.float32)
    for fi, sg in enumerate(F_SGS):
        for p in range(128):
            ri = sg * 128 + p
            if ri < NR:
                np.add.at(cnt[fi, p], tg_rot[ri], 1.0)
                add[fi, p, tg_rot[ri]] = 0.0
                assert tg_rot[ri, 0] < ROT
                dmk[fi, p, tg_rot[ri, 0]] = 1.0
            else:  # pad row: benign singleton at col 0
                cnt[fi, p, 0] = 1.0
                add[fi, p, 0] = 0.0
    return (
        cnt.astype(ml_dtypes.bfloat16),
        add.astype(ml_dtypes.bfloat16),
        dmk.astype(ml_dtypes.bfloat16),
    )


def _prep_in_maps(contexts, encodings, Wk_w, Wk_b, neg_idx):
    contexts = np.ascontiguousarray(np.asarray(contexts, np.float32))
    encodings = np.ascontiguousarray(np.asarray(encodings, np.float32))
    Wk_w = np.ascontiguousarray(np.asarray(Wk_w, np.float32))
    Wk_b = np.ascontiguousarray(np.asarray(Wk_b, np.float32))
    neg_idx = np.asarray(neg_idx)

    enc_flat = encodings.reshape(NT, D)
    ench = np.ascontiguousarray(enc_flat.astype(ml_dtypes.bfloat16))
    wkT = Wk_w.transpose(0, 2, 1)  # [S, d, e]
    wkTh = np.ascontiguousarray(wkT.astype(ml_dtypes.bfloat16))
    wkbH = np.ascontiguousarray(Wk_b[None, :, :].astype(ml_dtypes.bfloat16))

    in_maps = []
    for c in range(NCORES):
        bs = slice(c * BSH, (c + 1) * BSH)
        ctx_rows = np.concatenate(
            [contexts[bs, : 6 - s].reshape(-1, D) for s in range(S)], axis=0
        )
        ctxTh = ctx_rows.T.astype(ml_dtypes.bfloat16)
        enc_rot = np.roll(enc_flat, -c * ROT, axis=0)
        encTc = np.ascontiguousarray(enc_rot.T.astype(ml_dtypes.bfloat16))
        cnt, add, dmk = _build_masks(c, neg_idx)
        in_maps.append(
            {
                "ctxTh": np.ascontiguousarray(ctxTh),
                "wkTh": wkTh,
                "wkbH": wkbH,
                "ench": ench,
                "encT": encTc,
                "cmask": np.ascontiguousarray(cnt),
                "amask": np.ascontiguousarray(add),
                "dmask": np.ascontiguousarray(dmk),
                "idx": _build_idx(c, neg_idx),
            }
        )
    return in_maps


def kernel(contexts, encodings, Wk_w, Wk_b, neg_idx, _trace=False):
    in_maps = _prep_in_maps(contexts, encodings, Wk_w, Wk_b, neg_idx)
    nc = build_nc()
    res = run_bass_kernel_spmd(nc, in_maps, list(range(NCORES)), trace=_trace)
    LAST_RUN["exec_time_ns"] = res.exec_time_ns
    LAST_RUN["results"] = res.results
    loss = np.float32(0.0)
    corr = np.float32(0.0)
    for o in res.results:
        loss += np.float32(o["out"][0, 0])
        corr += np.float32(o["out"][0, 1])
    return (
        np.float32(loss / np.float32(N_PREDS)),
        np.float32(corr / np.float32(N_PREDS)),
    )


# revision 32
# speedup vs baseline: 1.3508x; 1.0132x over previous
"""Trainium2 Bass kernel for the CPC loss problem (nn_CPC_85117661872355).

Hybrid strategy (data-parallel over batch B across 8 cores; 1120 rows/core
packed s-major into 9 supergroups of 128 rows):

  Phase 1 (PE): predT[e, r] = sum_d Wk^T[d, e] * ctxT[d, r] as bf16 matmuls
  with the OUTPUT e-dim on partitions; the per-e bias is added in fp32 by the
  scalar engine during the PSUM->SBUF copy.  No repack DMA.

  Phase 2a "G" (supergroups 0-2): SWDGE row-gather of each row's 17 target
  vectors from a bf16 copy of the encoding table; per-row dots via DVE
  tensor_tensor_reduce against a row-major pred tile obtained by PE-transpose
  of predT.  Positives gathered through the same path keep exact ties.

  Phase 2b "F" (supergroups 3-8): full score matrix score = pred_sg @ encT
  on the PE against a per-core ROTATED table (rotation makes the positive
  column index core-independent, so one SPMD program works).  Per-row
  softmax stats come from host-built masks:
    - sum_sel exp = TTR(exp(score) * countmask, add)   (count = multiplicity,
      positive included, so duplicates and pos/neg collisions are exact)
    - max_sel     = TTR(score + addmask, max)          (addmask 0/-300)
    - l0 (positive logit) = diag-mask STT per (s,b) row-block
  All comparisons use the same bf16 score values, so argmax tie semantics
  match the reference exactly.

  Tail: loss = ln(ssum) - negm - pos (negm=0 for F), correct = pos >= maxs,
  both masked by row validity, partition-reduced by a ones-matmul; host sums
  the 8 [1,2] partials.
"""

import functools

import ml_dtypes
import numpy as np

import concourse.bass as bass
import concourse.mybir as mybir
import concourse.tile as tile
from concourse import bacc
from concourse.bass_utils import run_bass_kernel_spmd

F32 = mybir.dt.float32
BF16 = mybir.dt.bfloat16

B, G, D = 64, 7, 1280
S, NEG = 5, 16
NCORES = 8
BSH = B // NCORES  # 8
NT = B * G * G  # 3136 table rows
ROT = BSH * G * G  # 392: per-core table rotation
NS = [BSH * (6 - s) * G for s in range(S)]  # [336, 280, 224, 168, 112]
SOFF = [0]
for n in NS:
    SOFF.append(SOFF[-1] + n)
NR = SOFF[-1]  # 1120 rows per core
NRP = 1152  # padded to 9*128
NSG = 9
G_SGS = [0, 1, 2]
F_SGS = [3, 4, 5, 6, 7, 8]
NDOT = 17
GCHUNKS = [(0, 4), (4, 4), (8, 4), (12, 4), (16, 1)]  # (goff, width)
IDX_PER_SG = NDOT * 128  # 2176
IDX_TOT = len(G_SGS) * IDX_PER_SG
N_PREDS = B * G * 20  # 8960
NTILE = [(i * 512, min(512, NT - i * 512)) for i in range((NT + 511) // 512)]

# row blocks: contiguous (s, b) runs; (row_start, length, pos_col_base)
BLOCKS = []
for s in range(S):
    L = (6 - s) * G
    for b in range(BSH):
        BLOCKS.append((SOFF[s] + b * L, L, b * G * G + (s + 1) * G))


def _sg_blocks(sg):
    """[(pp0, n, cc0)] block slices within supergroup sg."""
    p_lo, p_hi = sg * 128, sg * 128 + 128
    out = []
    for ri0, L, c0 in BLOCKS:
        lo, hi = max(ri0, p_lo), min(ri0 + L, p_hi)
        if lo < hi:
            out.append((lo - p_lo, hi - lo, c0 + (lo - ri0)))
    return out


LAST_RUN = {}


@functools.lru_cache(maxsize=1)
def build_nc() -> bass.Bass:
    nc = bacc.Bacc(
        "TRN2",
        target_bir_lowering=False,
        debug=False,
        num_devices=NCORES,
    )
    ctxTh = nc.declare_dram_parameter("ctxTh", [D, NR], BF16, isOutput=False)
    wkTh = nc.declare_dram_parameter("wkTh", [S, D, D], BF16, isOutput=False)
    wkbH = nc.declare_dram_parameter("wkbH", [1, S, D], BF16, isOutput=False)
    ench = nc.declare_dram_parameter("ench", [NT, D], BF16, isOutput=False)
    encT = nc.declare_dram_parameter("encT", [D, NT], BF16, isOutput=False)
    cmask = nc.declare_dram_parameter("cmask", [len(F_SGS), 128, NT], BF16, isOutput=False)
    amask = nc.declare_dram_parameter("amask", [len(F_SGS), 128, NT], BF16, isOutput=False)
    dmask = nc.declare_dram_parameter("dmask", [len(F_SGS), 128, ROT], BF16, isOutput=False)
    idx = nc.declare_dram_parameter(
        "idx", [128, IDX_TOT // 16], mybir.dt.int16, isOutput=False
    )
    out = nc.declare_dram_parameter("out", [1, 2], F32, isOutput=True)

    Alu = mybir.AluOpType
    Act = mybir.ActivationFunctionType
    Ax = mybir.AxisListType

    with tile.TileContext(nc) as tc:
        with (
            tc.tile_pool(name="const", bufs=1) as constp,
            tc.tile_pool(name="wk", bufs=2) as wkp,
            tc.tile_pool(name="gath", bufs=2) as gathp,
            tc.tile_pool(name="predg", bufs=3) as predgp,
            tc.tile_pool(name="score", bufs=2) as scorep,
            tc.tile_pool(name="mask", bufs=1) as maskp,
            tc.tile_pool(name="scr", bufs=1) as scrp,
            tc.tile_pool(name="dots", bufs=2) as dotsp,
            tc.tile_pool(name="small", bufs=4) as smallp,
            tc.tile_pool(name="stage", bufs=2) as stagep,
            tc.tile_pool(name="psum", bufs=2, space="PSUM") as psump,
            tc.tile_pool(name="psumG", bufs=2, space="PSUM") as psumGp,
            tc.tile_pool(name="psumS", bufs=3, space="PSUM") as psumSp,
            tc.tile_pool(name="psumf", bufs=1, space="PSUM") as psumfp,
        ):
            # ---- constants ----
            idx_sb = constp.tile([128, IDX_TOT // 16], mybir.dt.int16, tag="idx")
            nc.sync.dma_start(idx_sb[:, :], idx[:, :])
            wkbh_sb = constp.tile([1, S, D], BF16, tag="wkbh")
            nc.sync.dma_start(wkbh_sb[:, :, :], wkbH[:, :, :])
            ones_sb = constp.tile([128, 1], F32, tag="ones")
            nc.vector.memset(ones_sb[:, :], 1.0)
            onesb16 = constp.tile([1, 512], BF16, tag="onesb16")
            nc.vector.memset(onesb16[:, :], 1.0)
            ctxh_sb = constp.tile([128, 10, NR], BF16, tag="ctxh")
            nc.sync.dma_start(
                ctxh_sb[:, :, :], ctxTh[:, :].rearrange("(dc di) r -> di dc r", di=128)
            )

            # CE partial columns
            negm_all = constp.tile([128, NSG], F32, tag="negm_all")
            ssum_all = constp.tile([128, NSG], F32, tag="ssum_all")
            pos_all = constp.tile([128, NSG], F32, tag="pos_all")
            maxs_all = constp.tile([128, NSG], F32, tag="maxs_all")
            nc.vector.memset(negm_all[:, :], 0.0)
            nc.vector.memset(pos_all[:, :], 0.0)
            vmask = constp.tile([128, NSG], F32, tag="vmask")
            nc.vector.memset(vmask[:, :], 1.0)
            nc.vector.memset(vmask[96:128, 8:9], 0.0)

            # transposed predictions (lhsT for F-path); pad cols zeroed
            predT = constp.tile([128, 10, NRP], BF16, tag="predT")
            nc.vector.memset(predT[:, :, NR:NRP], 0.0)
            pred_g = [
                predgp.tile([128, D], BF16, tag="predg", name=f"predg{i}")
                for i in G_SGS
            ]

            # ---- phase 1: predT = Wk^T-slices @ ctxT (+bias on copy) ----
            for s in range(S):
                wk_r = wkTh[s, :, :].rearrange("(dc di) e -> di dc e", di=128)
                for ch2 in range(5):  # 2 e-chunks of 128 per wk tile
                    wk_t = wkp.tile([128, 10, 256], BF16, tag="wk")
                    nc.sync.dma_start(
                        wk_t[:, :, :], wk_r[:, :, ch2 * 256 : ch2 * 256 + 256]
                    )
                    for e2 in range(2):
                        ec = ch2 * 2 + e2
                        ps = psump.tile([128, NS[s]], F32, tag="ps")
                        for dc in range(10):
                            nc.tensor.matmul(
                                ps[:, :],
                                lhsT=wk_t[:, dc, e2 * 128 : e2 * 128 + 128],
                                rhs=ctxh_sb[:, dc, SOFF[s] : SOFF[s] + NS[s]],
                                start=(dc == 0),
                                stop=False,
                            )
                        # bias: out[e, r] += wkb[e] * 1
                        nc.tensor.matmul(
                            ps[:, :],
                            lhsT=wkbh_sb[0:1, s, ec * 128 : ec * 128 + 128],
                            rhs=onesb16[0:1, : NS[s]],
                            start=False,
                            stop=True,
                        )
                        nc.scalar.copy(
                            predT[:, ec, SOFF[s] : SOFF[s] + NS[s]], ps[:, :]
                        )
                    # row-major pred for the G supergroups (rows 0..383):
                    # baseline-proven matmul + stage + repack-DMA pattern
                    for roff, M in {0: [(0, 128), (128, 128), (256, 80)],
                                    1: [(336, 48)]}.get(s, []):
                        psG = psumGp.tile([128, 256], F32, tag="psG")
                        for dc in range(10):
                            nc.tensor.matmul(
                                psG[:M, :],
                                lhsT=ctxh_sb[:, dc, roff : roff + M],
                                rhs=wk_t[:, dc, :],
                                start=(dc == 0),
                                stop=False,
                            )
                        nc.tensor.matmul(
                            psG[:M, :],
                            lhsT=onesb16[0:1, :M],
                            rhs=wkbh_sb[0:1, s, ch2 * 256 : ch2 * 256 + 256],
                            start=False,
                            stop=True,
                        )
                        stg = stagep.tile([128, 256], BF16, tag="stg")
                        nc.scalar.copy(stg[:M, :], psG[:M, :])
                        k, p0 = divmod(roff, 128)
                        n1 = min(M, 128 - p0)
                        nc.sync.dma_start(
                            pred_g[k][p0 : p0 + n1, ch2 * 256 : ch2 * 256 + 256],
                            stg[0:n1, :],
                        )
                        if M > n1:
                            nc.sync.dma_start(
                                pred_g[k + 1][0 : M - n1, ch2 * 256 : ch2 * 256 + 256],
                                stg[n1:M, :],
                            )

            encT_sb = constp.tile([128, 10, NT], BF16, tag="encT")
            nc.sync.dma_start(
                encT_sb[:, :, :], encT[:, :].rearrange("(dc di) n -> di dc n", di=128)
            )
            dmask_sb = constp.tile([128, len(F_SGS), ROT], BF16, tag="dmask")
            nc.sync.dma_start(
                dmask_sb[:, :, :], dmask[:, :, :].rearrange("f p c -> p f c")
            )
            # ---- phase 2a (G): gather + TTR dots + CE partials ----
            ench_ap = ench[:, :]
            for gi, sg in enumerate(G_SGS):
                dots_t = dotsp.tile([128, NDOT], F32, tag="dots")
                for goff, w in GCHUNKS:
                    gt = gathp.tile([128, 4, D], BF16, tag="gt")
                    pos0 = gi * IDX_PER_SG + goff * 128
                    nidx = w * 128
                    nc.gpsimd.dma_gather(
                        gt[:, :w, :],
                        ench_ap,
                        idx_sb[:, pos0 // 16 : (pos0 + nidx) // 16],
                        nidx,
                        nidx,
                        D,
                    )
                    for j in range(w):
                        scr = scrp.tile([128, NT], BF16, tag="fscr")
                        g = goff + j
                        nc.vector.scalar_tensor_tensor(
                            scr[:, 0:D],
                            gt[:, j, :],
                            1.0,
                            pred_g[gi][:, :],
                            op0=Alu.mult,
                            op1=Alu.mult,
                            accum_out=dots_t[:, g : g + 1],
                        )
                nc.vector.tensor_reduce(
                    negm_all[:, sg : sg + 1], dots_t[:, :], Ax.X, Alu.max, negate=True
                )
                e17 = smallp.tile([128, NDOT], F32, tag="e17")
                nc.scalar.activation(
                    e17[:, :],
                    dots_t[:, :],
                    Act.Exp,
                    bias=negm_all[:, sg : sg + 1],
                    scale=1.0,
                    accum_out=ssum_all[:, sg : sg + 1],
                )
                nc.vector.tensor_reduce(
                    maxs_all[:, sg : sg + 1], dots_t[:, 1:NDOT], Ax.X, Alu.max
                )
                nc.vector.tensor_copy(pos_all[:, sg : sg + 1], dots_t[:, 0:1])

            # ---- phase 2b (F): PE score matrix + mask reductions ----
            for fi, sg in enumerate(F_SGS):
                am = maskp.tile([128, NT], BF16, tag="am")
                nc.sync.dma_start(am[:, :], amask[fi, :, :])
                cm = maskp.tile([128, NT], BF16, tag="cm")
                nc.sync.dma_start(cm[:, :], cmask[fi, :, :])
                score = scorep.tile([128, NT], BF16, tag="score")
                for n0, nw in NTILE:
                    psS = psumSp.tile([128, 512], F32, tag="psS")
                    for dc in range(10):
                        nc.tensor.matmul(
                            psS[:, :nw],
                            lhsT=predT[:, dc, sg * 128 : sg * 128 + 128],
                            rhs=encT_sb[:, dc, n0 : n0 + nw],
                            start=(dc == 0),
                            stop=(dc == 9),
                        )
                    nc.scalar.copy(score[:, n0 : n0 + nw], psS[:, :nw])
                scrF = scrp.tile([128, NT], BF16, tag="fscr")
                nc.vector.tensor_tensor(scrF[:, :], score[:, :], am[:, :], Alu.add)
                nc.vector.tensor_reduce(
                    maxs_all[:, sg : sg + 1], scrF[:, :], Ax.X, Alu.max
                )
                e_t = scrp.tile([128, NT], BF16, tag="e_t")
                nc.scalar.activation(e_t[:, :], score[:, :], Act.Exp)
                nc.vector.scalar_tensor_tensor(
                    scrF[:, :],
                    e_t[:, :],
                    1.0,
                    cm[:, :],
                    op0=Alu.mult,
                    op1=Alu.mult,
                    accum_out=ssum_all[:, sg : sg + 1],
                )
                # positive logit: one-hot select over the [0, ROT) col window
                scrD = scrp.tile([128, NT], BF16, tag="fscr")
                nc.vector.scalar_tensor_tensor(
                    scrD[:, 0:ROT],
                    score[:, 0:ROT],
                    1.0,
                    dmask_sb[:, fi, :],
                    op0=Alu.mult,
                    op1=Alu.mult,
                    accum_out=pos_all[:, sg : sg + 1],
                )

            # ---- batched CE tail ----
            lns = smallp.tile([128, NSG], F32, tag="lns")
            nc.scalar.activation(lns[:, :], ssum_all[:, :], Act.Ln)
            loss_t = smallp.tile([128, NSG], F32, tag="loss_t")
            nc.vector.tensor_tensor(loss_t[:, :], lns[:, :], negm_all[:, :], Alu.subtract)
            nc.vector.tensor_tensor(loss_t[:, :], loss_t[:, :], pos_all[:, :], Alu.subtract)
            nc.vector.tensor_tensor(loss_t[:, :], loss_t[:, :], vmask[:, :], Alu.mult)
            acc2 = smallp.tile([128, 2], F32, tag="acc2")
            nc.vector.tensor_reduce(acc2[:, 0:1], loss_t[:, :], Ax.X, Alu.add)
            corr_t = smallp.tile([128, NSG], F32, tag="corr_t")
            nc.vector.tensor_tensor(corr_t[:, :], pos_all[:, :], maxs_all[:, :], Alu.is_ge)
            nc.vector.tensor_tensor(corr_t[:, :], corr_t[:, :], vmask[:, :], Alu.mult)
            nc.vector.tensor_reduce(acc2[:, 1:2], corr_t[:, :], Ax.X, Alu.add)

            psf = psumfp.tile([1, 2], F32, tag="psf")
            nc.tensor.matmul(
                psf[:, :], lhsT=ones_sb[:, 0:1], rhs=acc2[:, :], start=True, stop=True
            )
            outsb = smallp.tile([1, 2], F32, tag="outsb")
            nc.vector.tensor_copy(outsb[:, :], psf[:, :])
            nc.sync.dma_start(out[:, :], outsb[:, :])

    nc.compile()
    return nc


def _row_targets(core: int, neg_idx: np.ndarray, nrows: int) -> np.ndarray:
    """[nrows, 17] flat enc index (unrotated) of positive + negatives."""
    tg = np.zeros((nrows, NDOT), np.int64)
    ri = 0
    for s in range(S):
        rows = 6 - s
        for b in range(BSH):
            bg = core * BSH + b
            for r in range(rows):
                for c7 in range(G):
                    if ri >= nrows:
                        return tg
                    tg[ri, 0] = bg * G * G + (s + 1 + r) * G + c7
                    tg[ri, 1:] = neg_idx[bg, s, r, c7]
                    ri += 1
    return tg


def _build_idx(core: int, neg_idx: np.ndarray) -> np.ndarray:
    """int16 gather-index tensor (G supergroups only) in SWDGE wrap layout."""
    tg = _row_targets(core, neg_idx, len(G_SGS) * 128)
    lst = tg.reshape(len(G_SGS), 128, NDOT).transpose(0, 2, 1).reshape(-1)
    arr = lst.astype(np.int16).reshape(-1, 16).T  # [16, IDX_TOT//16]
    return np.ascontiguousarray(np.tile(arr, (8, 1)))


def _build_masks(core: int, neg_idx: np.ndarray):
    """count / additive masks [6, 128, NT] (rotated cols) + diag mask."""
    tg_all = _row_targets(core, neg_idx, NR)  # unrotated
    tg_rot = (tg_all - core * ROT) % NT
    cnt = np.zeros((len(F_SGS), 128, NT), np.float32)
    add = np.full((len(F_SGS), 128, NT), -300.0, np.float32)
    dmk = np.zeros((len(F_SGS), 128, ROT), np.float32)
    for fi, sg in enumerate(F_SGS):
        for p in range(128):
            ri = sg * 128 + p
            if ri < NR:
                np.add.at(cnt[fi, p], tg_rot[ri], 1.0)
                add[fi, p, tg_rot[ri]] = 0.0
                assert tg_rot[ri, 0] < ROT
                dmk[fi, p, tg_rot[ri, 0]] = 1.0
            else:  # pad row: benign singleton at col 0
                cnt[fi, p, 0] = 1.0
                add[fi, p, 0] = 0.0
    return (
        cnt.astype(ml_dtypes.bfloat16),
        add.astype(ml_dtypes.bfloat16),
        dmk.astype(ml_dtypes.bfloat16),
    )


def _prep_in_maps(contexts, encodings, Wk_w, Wk_b, neg_idx):
    contexts = np.ascontiguousarray(np.asarray(contexts, np.float32))
    encodings = np.ascontiguousarray(np.asarray(encodings, np.float32))
    Wk_w = np.ascontiguousarray(np.asarray(Wk_w, np.float32))
    Wk_b = np.ascontiguousarray(np.asarray(Wk_b, np.float32))
    neg_idx = np.asarray(neg_idx)

    enc_flat = encodings.reshape(NT, D)
    ench = np.ascontiguousarray(enc_flat.astype(ml_dtypes.bfloat16))
    wkT = Wk_w.transpose(0, 2, 1)  # [S, d, e]
    wkTh = np.ascontiguousarray(wkT.astype(ml_dtypes.bfloat16))
    wkbH = np.ascontiguousarray(Wk_b[None, :, :].astype(ml_dtypes.bfloat16))

    in_maps = []
    for c in range(NCORES):
        bs = slice(c * BSH, (c + 1) * BSH)
        ctx_rows = np.concatenate(
            [contexts[bs, : 6 - s].reshape(-1, D) for s in range(S)], axis=0
        )
        ctxTh = ctx_rows.T.astype(ml_dtypes.bfloat16)
        enc_rot = np.roll(enc_flat, -c * ROT, axis=0)
        encTc = np.ascontiguousarray(enc_rot.T.astype(ml_dtypes.bfloat16))
        cnt, add, dmk = _build_masks(c, neg_idx)
        in_maps.append(
            {
                "ctxTh": np.ascontiguousarray(ctxTh),
                "wkTh": wkTh,
                "wkbH": wkbH,
                "ench": ench,
                "encT": encTc,
                "cmask": np.ascontiguousarray(cnt),
                "amask": np.ascontiguousarray(add),
                "dmask": np.ascontiguousarray(dmk),
                "idx": _build_idx(c, neg_idx),
            }
        )
    return in_maps


def kernel(contexts, encodings, Wk_w, Wk_b, neg_idx, _trace=False):
    in_maps = _prep_in_maps(contexts, encodings, Wk_w, Wk_b, neg_idx)
    nc = build_nc()
    res = run_bass_kernel_spmd(nc, in_maps, list(range(NCORES)), trace=_trace)
    LAST_RUN["exec_time_ns"] = res.exec_time_ns
    LAST_RUN["results"] = res.results
    loss = np.float32(0.0)
    corr = np.float32(0.0)
    for o in res.results:
        loss += np.float32(o["out"][0, 0])
        corr += np.float32(o["out"][0, 1])
    return (
        np.float32(loss / np.float32(N_PREDS)),
        np.float32(corr / np.float32(N_PREDS)),
    )
